# revision 46
# baseline (speedup 1.0000x reference)
"""Trainium2 Bass kernel for nn_CACProjector (logits = x @ W^T, CAC distances).

Strategy: data-parallel over batch B across 8 NeuronCores. Each core gets a
(768, 2048) column-slice xT of x^T (host-side transpose so the contraction
dim D lands on SBUF partitions) and a replicated W^T (768, 1024). On-core:

  logits[b, c] = sum_d xT[d, b] * wT[d, c]      (PE, fp32 accumulate in PSUM)
  sq_norm[b]   = sum_c logits[b, c]^2 + alpha^2 (DVE square+reduce+add on the
                                                 bf16 logits copy)
  dist[b, c]   = sqrt(sq_norm[b] - 2*alpha*logits[b, c])
                                                 (ACT Sqrt w/ scale+bias)

The PE runs at its bf16 roofline (192 N=512 matmuls x ~216 ns = 41.5 us);
everything else is scheduled to hide under it:

- All input loads ride the sync HWDGE ring in exact consumption order
  (k0 pieces split small so the first real matmul starts ~10.5 us).
  Splitting loads across both HWDGE rings was measured WORSE: the SDMA
  engine pool round-robins between rings, so parallel issue just halves
  per-transfer bandwidth and delays every early completion.
- The k-major warm-in groups are 2 b-tiles, so the first epilogues fire at
  ~17 us and the ACT/DVE epilogue work (~34 us each side) fits inside the
  matmul window instead of backlogging past its end.
- Per-tile epilogue: one engine copies PSUM->SBUF bf16 (the sole PSUM
  reader; DVE for DVE_COPY_TILES, else ACT), then DVE squares+reduces the
  copy (tensor_tensor_reduce crashes this fabric's exec unit - don't), then
  ACT Sqrt. PSUM banks free right at the copy, so the 4-buffer pool never
  stalls the PE.
- End of kernel: tile 14's sqrt is deferred in ACT program order past tile
  15's lo-half copy (the in-order ACT queue would otherwise head-of-line
  block the last chain); tile 15's copy halves run on ACT+DVE concurrently
  (different PSUM banks), and its dist halves store via both DMA rings to
  overlap the ~1.5 us store-receipt latency.
- 6 dummy matmuls on a zeroed tile (memset on GpSimd, whose preamble ends
  earliest) bridge NEFF-preamble-end (~7.3 us) to first-input-landed
  (~10.5 us) so the PE's HAM clock-gate (1.2 -> 2.4 GHz after ~3.4 us of
  sustained activity) is released when the real stream starts. A PE idle
  gap > ~1 us here re-throttles the clock for ~4 us of the stream.

d2 = ||l||^2 - 2a*l_j + a^2 >= (l_j - a)^2 >= 0 mathematically, and with this
data d2 ~ 1100 >> 0, so the reference's maximum(d2, 0) clamp is a no-op.

IO_MODE picks transport precision ("bf16" ships x/W/outputs bf16, fp32 PSUM
accumulate; "f32r" ships fp32 rounded to TF32).
"""

import sys

sys.path.insert(0, "/opt/trn_rl_repo")

from contextlib import ExitStack

import ml_dtypes
import numpy as np

import concourse.tile as tile
from concourse import bacc, mybir
from concourse.bass_utils import run_bass_kernel_spmd

N_CORES = 8
B, D, C = 16384, 768, 1024
BS = B // N_CORES          # 2048 rows of B per core
P = 128                    # partition dim
KT = D // P                # 6 contraction chunks
NBT = BS // P              # 16 output row-tiles per core
ALPHA = 10.0

F32 = mybir.dt.float32
F32R = mybir.dt.float32r
BF16 = mybir.dt.bfloat16

IO_MODE = "bf16"
# 6 dummies bridge preamble-end (~7.16us) toward first-input-landed
# (~10.1-10.6). A 7th (to close the residual 0.4us gap for HAM-window
# insurance) measured no better and delays the real stream when inputs land
# early — 6 won on measurement (64.5us vs 65.5).
N_DUMMY = 6
# tiles whose PSUM->SBUF copy runs on DVE (rest: ACT) — balances ACT/DVE.
# With USE_STT freeing ~0.6us/tile of DVE, six DVE copies pull ACT from
# ~95% occupancy (35.5us in a 38us window — the real epilogue constraint)
# down to ~30us, giving both engines slack.
DVE_COPY_TILES = (1, 3, 5, 7, 9, 11)
# tiles whose square runs on GpSimd: measured 1.9us/op (2.7x DVE) and it
# serializes into every tile's chain — keep empty.
GPSIMD_SQ_TILES = frozenset()
# fused square+row-sum via scalar_tensor_tensor (one DVE pass instead of
# tensor_tensor + tensor_reduce): SCALAR_TENSOR_TENSOR 1224ns + 84ns
# DVE_READ_ACCUMULATOR, saving ~0.6us/tile of DVE time. Alone it bought
# nothing (ACT was the saturated engine); combined with the extra DVE
# copies above it unloads ACT, which is the point.
USE_STT = True

# Feature flags (bisectable): scalar-ring DMA issue, fused square+reduce,
# DVE-side PSUM copies.
SCALAR_DMA = True
USE_TTR = False  # tensor_tensor_reduce crashes the exec unit on this fabric
DVE_COPY = True


def build(io_mode=IO_MODE, scalar_dma=None, use_ttr=None, dve_copy=None):
    scalar_dma = SCALAR_DMA if scalar_dma is None else scalar_dma
    use_ttr = USE_TTR if use_ttr is None else use_ttr
    dve_copy = DVE_COPY if dve_copy is None else dve_copy
    in_dt = BF16 if io_mode == "bf16" else F32R
    out_dt = BF16 if io_mode == "bf16" else F32
    MULT = mybir.AluOpType.mult
    ADD = mybir.AluOpType.add

    nc = bacc.Bacc("TRN2", target_bir_lowering=False, debug=False)
    xT = nc.dram_tensor("xT", [D, BS], in_dt, kind="ExternalInput").ap()
    wT = nc.dram_tensor("wT", [D, C], in_dt, kind="ExternalInput").ap()
    logits = nc.dram_tensor("logits", [BS, C], out_dt, kind="ExternalOutput").ap()
    dist = nc.dram_tensor("dist", [BS, C], out_dt, kind="ExternalOutput").ap()

    with tile.TileContext(nc) as tc, ExitStack() as ctx:
        xpool = ctx.enter_context(tc.tile_pool(name="xT", bufs=1))
        wpool = ctx.enter_context(tc.tile_pool(name="wT", bufs=1))
        psum = ctx.enter_context(tc.tile_pool(name="psum", bufs=4, space="PSUM"))
        lpool = ctx.enter_context(tc.tile_pool(name="lg", bufs=4))
        dpool = ctx.enter_context(tc.tile_pool(name="dist", bufs=4))
        spool = ctx.enter_context(tc.tile_pool(name="sq", bufs=2))
        npool = ctx.enter_context(tc.tile_pool(name="norms", bufs=4))

        # PE warmup fodder (no DMA dependency). memset on GpSimd: its
        # preamble ends ~6.3us vs Vector's ~7.0, so the dummy matmuls (and
        # with them the HAM clock-gate warmup window) start ~0.6us earlier.
        warm = xpool.tile([P, 512], in_dt, tag="warm")
        nc.gpsimd.memset(warm[:], 0)

        # ---- input loads, split across the two HWDGE rings ----
        # scalar (ACT) ring: x for b-tiles 0-3, then x for b-tiles 8-15
        # All input loads ride the sync ring only, in consumption order:
        # parallel-ring issue just splits SDMA bandwidth and delays every
        # early completion (measured: k1 pieces took 4.8us issue->complete
        # with two rings vs ~1us cadence single-ring). First k0 pieces are
        # small so the first real matmul can start ~10.3us.
        x_eng = nc.scalar if scalar_dma else nc.sync
        xA0 = xpool.tile([P, 512], in_dt, tag="xA0")
        nc.sync.dma_start(xA0[:, 0:256], xT[0:P, 0:256])
        w0lo = wpool.tile([P, 512], in_dt, tag="w0lo")
        nc.sync.dma_start(w0lo[:], wT[0:P, 0:512])
        w0hi = wpool.tile([P, 512], in_dt, tag="w0hi")
        nc.sync.dma_start(w0hi[:], wT[0:P, 512:1024])
        nc.sync.dma_start(xA0[:, 256:512], xT[0:P, 256:512])
        xA = [xA0]
        wt_lo, wt_hi = [w0lo[:]], [w0hi[:]]
        for k in range(1, KT):
            t = xpool.tile([P, 512], in_dt, tag=f"xA{k}")
            nc.sync.dma_start(t[:], xT[k * P : (k + 1) * P, 0:512])
            xA.append(t)
            wk = wpool.tile([P, C], in_dt, tag=f"w{k}")
            nc.sync.dma_start(wk[:], wT[k * P : (k + 1) * P, :])
            wt_lo.append(wk[:, 0:512])
            wt_hi.append(wk[:, 512:1024])
        xB = []
        for k in range(KT):
            t = xpool.tile([P, 512], in_dt, tag=f"xB{k}")
            nc.sync.dma_start(t[:], xT[k * P : (k + 1) * P, 512:1024])
            xB.append(t)
        # xC stays on the sync ring too. Moving it to the scalar ring was
        # tried with padding ops to delay its transfers past the critical
        # ramp window — but Tile schedules by dependency/priority, not
        # emission order, so the dependency-free xC issues ran FIRST and
        # contended with the ramp (+6us). Two concurrent rings halve each
        # transfer's bandwidth (SDMA packet round-robin).
        xC = []
        for k in range(KT):
            t = xpool.tile([P, 1024], in_dt, tag=f"xC{k}")
            nc.sync.dma_start(t[:], xT[k * P : (k + 1) * P, 1024:2048])
            xC.append(t)

        def x_slice(k, bt):
            if bt < 4:
                return xA[k][:, bt * P : (bt + 1) * P]
            if bt < 8:
                return xB[k][:, (bt - 4) * P : (bt - 3) * P]
            return xC[k][:, (bt - 8) * P : (bt - 7) * P]

        def mm(bt, ps, k):
            lhs = x_slice(k, bt)
            nc.tensor.matmul(
                ps[:, 0:512], lhs, wt_lo[k], start=(k == 0), stop=(k == KT - 1)
            )
            nc.tensor.matmul(
                ps[:, 512:1024], lhs, wt_hi[k], start=(k == 0), stop=(k == KT - 1)
            )

        def epi_copy(bt, ps):
            # The copy is the sole PSUM reader; DVE for tiles in
            # DVE_COPY_TILES (load balance + burst parallelism), ACT else.
            lg = lpool.tile([P, C], out_dt, name=f"lg{bt}")
            if dve_copy and bt in DVE_COPY_TILES:
                nc.vector.tensor_copy(lg[:], ps[:])
            else:
                nc.scalar.copy(lg[:], ps[:])
            nc.sync.dma_start(logits[bt * P : (bt + 1) * P, :], lg[:])
            return lg

        def epi_norm(bt, lg):
            # DVE: square + reduce + (+alpha^2) -> snb. scalar_tensor_tensor
            # fuses square+row-sum into ONE pass (InstTensorScalarPtr — a
            # different opcode family from the broken tensor_tensor_reduce).
            sq = spool.tile([P, C], out_dt, tag="sq")
            sn = npool.tile([P, 1], F32, tag="sn")
            snb = npool.tile([P, 1], F32, tag="snb")
            if USE_STT:
                nc.vector.scalar_tensor_tensor(
                    sq[:], lg[:], 1.0, lg[:], mybir.AluOpType.bypass, MULT,
                    accum_out=sn[:],
                )
            else:
                sq_eng = nc.gpsimd if bt in GPSIMD_SQ_TILES else nc.vector
                sq_eng.tensor_tensor(sq[:], lg[:], lg[:], MULT)
                nc.vector.tensor_reduce(
                    sn[:], sq[:], axis=mybir.AxisListType.X, op=ADD
                )
            nc.vector.tensor_scalar_add(snb[:], sn[:], ALPHA * ALPHA)
            return snb

        def epi_sqrt(bt, lg, snb):
            dt_ = dpool.tile([P, C], out_dt)
            nc.scalar.activation(
                dt_[:],
                lg[:],
                mybir.ActivationFunctionType.Sqrt,
                bias=snb[:],
                scale=-2.0 * ALPHA,
            )
            nc.sync.dma_start(dist[bt * P : (bt + 1) * P, :], dt_[:])

        def epi_chain(bt, lg):
            epi_sqrt(bt, lg, epi_norm(bt, lg))

        def epilogue(bt, ps):
            epi_chain(bt, epi_copy(bt, ps))

        def epilogue_last(bt, ps, pending):
            # End-of-kernel chain. Tile 14's sqrt was deferred (pending) so
            # ACT can run this tile's lo-half copy the moment the last matmul
            # retires (no head-of-line blocking behind sqrt14's DVE wait);
            # DVE copies the hi half concurrently (different PSUM bank). The
            # dist halves store via both DMA rings to overlap the final
            # receipt latency.
            lg = lpool.tile([P, C], out_dt, name=f"lg{bt}")
            nc.scalar.copy(lg[:, 0:512], ps[:, 0:512])
            nc.vector.tensor_copy(lg[:, 512:1024], ps[:, 512:1024])
            nc.sync.dma_start(logits[bt * P : (bt + 1) * P, :], lg[:])
            if pending is not None:
                epi_sqrt(*pending)

            snb = epi_norm(bt, lg)

            dt_ = dpool.tile([P, C], out_dt)
            nc.scalar.activation(
                dt_[:, 0:512],
                lg[:, 0:512],
                mybir.ActivationFunctionType.Sqrt,
                bias=snb[:],
                scale=-2.0 * ALPHA,
            )
            nc.sync.dma_start(dist[bt * P : (bt + 1) * P, 0:512], dt_[:, 0:512])
            nc.scalar.activation(
                dt_[:, 512:1024],
                lg[:, 512:1024],
                mybir.ActivationFunctionType.Sqrt,
                bias=snb[:],
                scale=-2.0 * ALPHA,
            )
            x_eng.dma_start(
                dist[bt * P : (bt + 1) * P, 512:1024], dt_[:, 512:1024]
            )

        # ---- matmul schedule ----
        pss = [psum.tile([P, C], F32, tag="ps", name=f"ps{i}") for i in range(4)]

        # Dummy matmuls bridge preamble-end -> first-input-landed so HAM's
        # clock-gate releases before the real stream starts. They land in
        # pss[3]'s hi bank; b-tile 3's start=True group overwrites later.
        for _ in range(N_DUMMY):
            nc.tensor.matmul(
                pss[3][:, 512:1024],
                warm[:, 0:P],
                warm[:],
                start=True,
                stop=True,
                skip_group_check=True,
            )

        # k-major warm-in groups of 2 b-tiles: each (x_k, w_k) DMA piece
        # unlocks 4 matmuls, and the first epilogues fire early enough that
        # ACT/DVE epilogue work fits inside the matmul window. Both copies of
        # a group are emitted before the chains so ACT and DVE drain the two
        # PSUM buffers concurrently.
        for g0 in (0, 2):
            for k in range(KT):
                if g0 == 0 and k == 0:
                    # k0 runs lo-halves of both tiles first: they need only
                    # the first two DMA pieces (xA0's first half + w0lo), so
                    # the real stream starts ~1us before w0hi lands. Costs 2
                    # extra LDWEIGHTS.
                    for i in (0, 1):
                        nc.tensor.matmul(
                            pss[i][:, 0:512], x_slice(0, i), wt_lo[0],
                            start=True, stop=False,
                        )
                    for i in (0, 1):
                        nc.tensor.matmul(
                            pss[i][:, 512:1024], x_slice(0, i), wt_hi[0],
                            start=True, stop=False,
                        )
                    continue
                for i in (g0, g0 + 1):
                    mm(i, pss[i], k)
            lgs = [epi_copy(g0 + i, pss[g0 + i]) for i in range(2)]
            for i in range(2):
                epi_chain(g0 + i, lgs[i])

        pending = None
        for bt in range(4, NBT):
            ps = psum.tile([P, C], F32, tag="ps")
            for k in range(KT):
                mm(bt, ps, k)
            if not dve_copy:
                epilogue(bt, ps)
            elif bt == NBT - 2:
                # defer tile 14's sqrt past tile 15's copies (ACT in-order
                # queue would otherwise block the last tile's chain start)
                lg = epi_copy(bt, ps)
                pending = (bt, lg, epi_norm(bt, lg))
            elif bt == NBT - 1:
                epilogue_last(bt, ps, pending)
            else:
                epilogue(bt, ps)

    nc.compile()
    return nc


_NC = {}


def _round_tf32(a):
    """Round-to-nearest-even to TF32 (10-bit mantissa) in fp32 storage."""
    u = a.view(np.uint32)
    r = (u + np.uint32(0xFFF) + ((u >> np.uint32(13)) & np.uint32(1))) & np.uint32(
        0xFFFFE000
    )
    return r.view(np.float32)


def kernel(x, W, trace=False, _result_box=None, io_mode=IO_MODE, **flags):
    key = (io_mode, tuple(sorted(flags.items())))
    if key not in _NC:
        _NC[key] = build(io_mode, **flags)
    nc = _NC[key]

    x = np.ascontiguousarray(np.asarray(x, dtype=np.float32))
    W = np.ascontiguousarray(np.asarray(W, dtype=np.float32))
    if io_mode == "bf16":
        prep = lambda a: np.asarray(a, dtype=ml_dtypes.bfloat16)
    else:
        prep = _round_tf32
    wT = prep(np.ascontiguousarray(W.T))
    in_maps = [
        {
            "xT": prep(np.ascontiguousarray(x[i * BS : (i + 1) * BS, :].T)),
            "wT": wT,
        }
        for i in range(N_CORES)
    ]

    # The first execution of a freshly loaded NEFF has been seen to flake
    # (transient NRT_EXEC_UNIT_UNRECOVERABLE / corrupt output on this
    # fabric); do a throwaway warm-up exec with one retry, then the real run.
    try:
        run_bass_kernel_spmd(nc, in_maps, list(range(N_CORES)))
    except Exception:
        try:
            run_bass_kernel_spmd(nc, in_maps, list(range(N_CORES)))
        except Exception:
            pass

    res = run_bass_kernel_spmd(nc, in_maps, list(range(N_CORES)), trace=trace)
    if _result_box is not None:
        _result_box.append(res)

    logits = np.concatenate(
        [np.asarray(res.results[i]["logits"], dtype=np.float32) for i in range(N_CORES)],
        axis=0,
    )
    dist = np.concatenate(
        [np.asarray(res.results[i]["dist"], dtype=np.float32) for i in range(N_CORES)],
        axis=0,
    )
    return logits, dist


# revision 48
# speedup vs baseline: 1.0333x; 1.0333x over previous
"""Trainium2 Bass kernel for nn_CACProjector (logits = x @ W^T, CAC distances).

Strategy: data-parallel over batch B across 8 NeuronCores. Each core gets a
(768, 2048) column-slice xT of x^T (host-side transpose so the contraction
dim D lands on SBUF partitions) and a replicated W^T (768, 1024). On-core:

  logits[b, c] = sum_d xT[d, b] * wT[d, c]      (PE, fp32 accumulate in PSUM)
  sq_norm[b]   = sum_c logits[b, c]^2 + alpha^2 (DVE square+reduce+add on the
                                                 bf16 logits copy)
  dist[b, c]   = sqrt(sq_norm[b] - 2*alpha*logits[b, c])
                                                 (ACT Sqrt w/ scale+bias)

The PE runs at its bf16 roofline (192 N=512 matmuls x ~216 ns = 41.5 us);
everything else is scheduled to hide under it:

- All input loads ride the sync HWDGE ring in exact consumption order
  (k0 pieces split small so the first real matmul starts ~10.5 us).
  Splitting loads across both HWDGE rings was measured WORSE: the SDMA
  engine pool round-robins between rings, so parallel issue just halves
  per-transfer bandwidth and delays every early completion.
- The k-major warm-in groups are 2 b-tiles, so the first epilogues fire at
  ~17 us and the ACT/DVE epilogue work (~34 us each side) fits inside the
  matmul window instead of backlogging past its end.
- Per-tile epilogue: one engine copies PSUM->SBUF bf16 (the sole PSUM
  reader; DVE for DVE_COPY_TILES, else ACT), then DVE squares+reduces the
  copy (tensor_tensor_reduce crashes this fabric's exec unit - don't), then
  ACT Sqrt. PSUM banks free right at the copy, so the 4-buffer pool never
  stalls the PE.
- End of kernel: tile 14's sqrt is deferred in ACT program order past tile
  15's lo-half copy (the in-order ACT queue would otherwise head-of-line
  block the last chain); tile 15's copy halves run on ACT+DVE concurrently
  (different PSUM banks), and its dist halves store via both DMA rings to
  overlap the ~1.5 us store-receipt latency.
- 6 dummy matmuls on a zeroed tile (memset on GpSimd, whose preamble ends
  earliest) bridge NEFF-preamble-end (~7.3 us) to first-input-landed
  (~10.5 us) so the PE's HAM clock-gate (1.2 -> 2.4 GHz after ~3.4 us of
  sustained activity) is released when the real stream starts. A PE idle
  gap > ~1 us here re-throttles the clock for ~4 us of the stream.

d2 = ||l||^2 - 2a*l_j + a^2 >= (l_j - a)^2 >= 0 mathematically, and with this
data d2 ~ 1100 >> 0, so the reference's maximum(d2, 0) clamp is a no-op.

IO_MODE picks transport precision ("bf16" ships x/W/outputs bf16, fp32 PSUM
accumulate; "f32r" ships fp32 rounded to TF32).
"""

import sys

sys.path.insert(0, "/opt/trn_rl_repo")

from contextlib import ExitStack

import ml_dtypes
import numpy as np

import concourse.tile as tile
from concourse import bacc, mybir
from concourse.bass_utils import run_bass_kernel_spmd

N_CORES = 8
B, D, C = 16384, 768, 1024
BS = B // N_CORES          # 2048 rows of B per core
P = 128                    # partition dim
KT = D // P                # 6 contraction chunks
NBT = BS // P              # 16 output row-tiles per core
ALPHA = 10.0

F32 = mybir.dt.float32
F32R = mybir.dt.float32r
BF16 = mybir.dt.bfloat16

IO_MODE = "bf16"
# 6 dummies bridge preamble-end (~7.16us) toward first-input-landed
# (~10.1-10.6). A 7th (to close the residual 0.4us gap for HAM-window
# insurance) measured no better and delays the real stream when inputs land
# early — 6 won on measurement (64.5us vs 65.5).
N_DUMMY = 6
# tiles whose PSUM->SBUF copy runs on DVE (rest: ACT) — balances ACT/DVE.
# (1,3) measured best. STT + six DVE copies (to unload ~95%-occupied ACT)
# measured 67.1us vs ~64.5 for this config — reverted.
DVE_COPY_TILES = (1, 3)
# tiles whose square runs on GpSimd: measured 1.9us/op (2.7x DVE) and it
# serializes into every tile's chain — keep empty.
GPSIMD_SQ_TILES = frozenset()
# fused square+row-sum via scalar_tensor_tensor (one DVE pass instead of
# tensor_tensor + tensor_reduce): SCALAR_TENSOR_TENSOR 1224ns + 84ns
# DVE_READ_ACCUMULATOR, correct on HW, saves ~0.6us/tile of DVE occupancy —
# but measured WORSE end-to-end both alone (66.7us) and with rebalanced
# copies (67.1us) vs the unfused program (64.0-65.5us). Keep False.
USE_STT = False

# Feature flags (bisectable): scalar-ring DMA issue, fused square+reduce,
# DVE-side PSUM copies.
SCALAR_DMA = True
USE_TTR = False  # tensor_tensor_reduce crashes the exec unit on this fabric
DVE_COPY = True


def build(io_mode=IO_MODE, scalar_dma=None, use_ttr=None, dve_copy=None):
    scalar_dma = SCALAR_DMA if scalar_dma is None else scalar_dma
    use_ttr = USE_TTR if use_ttr is None else use_ttr
    dve_copy = DVE_COPY if dve_copy is None else dve_copy
    in_dt = BF16 if io_mode == "bf16" else F32R
    out_dt = BF16 if io_mode == "bf16" else F32
    MULT = mybir.AluOpType.mult
    ADD = mybir.AluOpType.add

    nc = bacc.Bacc("TRN2", target_bir_lowering=False, debug=False)
    xT = nc.dram_tensor("xT", [D, BS], in_dt, kind="ExternalInput").ap()
    wT = nc.dram_tensor("wT", [D, C], in_dt, kind="ExternalInput").ap()
    logits = nc.dram_tensor("logits", [BS, C], out_dt, kind="ExternalOutput").ap()
    dist = nc.dram_tensor("dist", [BS, C], out_dt, kind="ExternalOutput").ap()

    with tile.TileContext(nc) as tc, ExitStack() as ctx:
        xpool = ctx.enter_context(tc.tile_pool(name="xT", bufs=1))
        wpool = ctx.enter_context(tc.tile_pool(name="wT", bufs=1))
        psum = ctx.enter_context(tc.tile_pool(name="psum", bufs=4, space="PSUM"))
        lpool = ctx.enter_context(tc.tile_pool(name="lg", bufs=4))
        dpool = ctx.enter_context(tc.tile_pool(name="dist", bufs=4))
        spool = ctx.enter_context(tc.tile_pool(name="sq", bufs=2))
        npool = ctx.enter_context(tc.tile_pool(name="norms", bufs=4))

        # PE warmup fodder (no DMA dependency). memset on GpSimd: its
        # preamble ends ~6.3us vs Vector's ~7.0, so the dummy matmuls (and
        # with them the HAM clock-gate warmup window) start ~0.6us earlier.
        warm = xpool.tile([P, 512], in_dt, tag="warm")
        nc.gpsimd.memset(warm[:], 0)

        # ---- input loads, split across the two HWDGE rings ----
        # scalar (ACT) ring: x for b-tiles 0-3, then x for b-tiles 8-15
        # All input loads ride the sync ring only, in consumption order:
        # parallel-ring issue just splits SDMA bandwidth and delays every
        # early completion (measured: k1 pieces took 4.8us issue->complete
        # with two rings vs ~1us cadence single-ring). First k0 pieces are
        # small so the first real matmul can start ~10.3us.
        x_eng = nc.scalar if scalar_dma else nc.sync
        xA0 = xpool.tile([P, 512], in_dt, tag="xA0")
        nc.sync.dma_start(xA0[:, 0:256], xT[0:P, 0:256])
        w0lo = wpool.tile([P, 512], in_dt, tag="w0lo")
        nc.sync.dma_start(w0lo[:], wT[0:P, 0:512])
        w0hi = wpool.tile([P, 512], in_dt, tag="w0hi")
        nc.sync.dma_start(w0hi[:], wT[0:P, 512:1024])
        nc.sync.dma_start(xA0[:, 256:512], xT[0:P, 256:512])
        xA = [xA0]
        wt_lo, wt_hi = [w0lo[:]], [w0hi[:]]
        for k in range(1, KT):
            t = xpool.tile([P, 512], in_dt, tag=f"xA{k}")
            nc.sync.dma_start(t[:], xT[k * P : (k + 1) * P, 0:512])
            xA.append(t)
            wk = wpool.tile([P, C], in_dt, tag=f"w{k}")
            nc.sync.dma_start(wk[:], wT[k * P : (k + 1) * P, :])
            wt_lo.append(wk[:, 0:512])
            wt_hi.append(wk[:, 512:1024])
        xB = []
        for k in range(KT):
            t = xpool.tile([P, 512], in_dt, tag=f"xB{k}")
            nc.sync.dma_start(t[:], xT[k * P : (k + 1) * P, 512:1024])
            xB.append(t)
        # xC stays on the sync ring too. Moving it to the scalar ring was
        # tried with padding ops to delay its transfers past the critical
        # ramp window — but Tile schedules by dependency/priority, not
        # emission order, so the dependency-free xC issues ran FIRST and
        # contended with the ramp (+6us). Two concurrent rings halve each
        # transfer's bandwidth (SDMA packet round-robin).
        xC = []
        for k in range(KT):
            t = xpool.tile([P, 1024], in_dt, tag=f"xC{k}")
            nc.sync.dma_start(t[:], xT[k * P : (k + 1) * P, 1024:2048])
            xC.append(t)

        def x_slice(k, bt):
            if bt < 4:
                return xA[k][:, bt * P : (bt + 1) * P]
            if bt < 8:
                return xB[k][:, (bt - 4) * P : (bt - 3) * P]
            return xC[k][:, (bt - 8) * P : (bt - 7) * P]

        def mm(bt, ps, k):
            lhs = x_slice(k, bt)
            nc.tensor.matmul(
                ps[:, 0:512], lhs, wt_lo[k], start=(k == 0), stop=(k == KT - 1)
            )
            nc.tensor.matmul(
                ps[:, 512:1024], lhs, wt_hi[k], start=(k == 0), stop=(k == KT - 1)
            )

        def epi_copy(bt, ps):
            # The copy is the sole PSUM reader; DVE for tiles in
            # DVE_COPY_TILES (load balance + burst parallelism), ACT else.
            lg = lpool.tile([P, C], out_dt, name=f"lg{bt}")
            if dve_copy and bt in DVE_COPY_TILES:
                nc.vector.tensor_copy(lg[:], ps[:])
            else:
                nc.scalar.copy(lg[:], ps[:])
            nc.sync.dma_start(logits[bt * P : (bt + 1) * P, :], lg[:])
            return lg

        def epi_norm(bt, lg):
            # DVE: square + reduce + (+alpha^2) -> snb. scalar_tensor_tensor
            # fuses square+row-sum into ONE pass (InstTensorScalarPtr — a
            # different opcode family from the broken tensor_tensor_reduce).
            sq = spool.tile([P, C], out_dt, tag="sq")
            sn = npool.tile([P, 1], F32, tag="sn")
            snb = npool.tile([P, 1], F32, tag="snb")
            if USE_STT:
                nc.vector.scalar_tensor_tensor(
                    sq[:], lg[:], 1.0, lg[:], mybir.AluOpType.bypass, MULT,
                    accum_out=sn[:],
                )
            else:
                sq_eng = nc.gpsimd if bt in GPSIMD_SQ_TILES else nc.vector
                sq_eng.tensor_tensor(sq[:], lg[:], lg[:], MULT)
                nc.vector.tensor_reduce(
                    sn[:], sq[:], axis=mybir.AxisListType.X, op=ADD
                )
            nc.vector.tensor_scalar_add(snb[:], sn[:], ALPHA * ALPHA)
            return snb

        def epi_sqrt(bt, lg, snb):
            dt_ = dpool.tile([P, C], out_dt)
            nc.scalar.activation(
                dt_[:],
                lg[:],
                mybir.ActivationFunctionType.Sqrt,
                bias=snb[:],
                scale=-2.0 * ALPHA,
            )
            nc.sync.dma_start(dist[bt * P : (bt + 1) * P, :], dt_[:])

        def epi_chain(bt, lg):
            epi_sqrt(bt, lg, epi_norm(bt, lg))

        def epilogue(bt, ps):
            epi_chain(bt, epi_copy(bt, ps))

        def epilogue_last(bt, ps, pending):
            # End-of-kernel chain. Tile 14's sqrt was deferred (pending) so
            # ACT can run this tile's lo-half copy the moment the last matmul
            # retires (no head-of-line blocking behind sqrt14's DVE wait);
            # DVE copies the hi half concurrently (different PSUM bank). The
            # dist halves store via both DMA rings to overlap the final
            # receipt latency.
            lg = lpool.tile([P, C], out_dt, name=f"lg{bt}")
            nc.scalar.copy(lg[:, 0:512], ps[:, 0:512])
            nc.vector.tensor_copy(lg[:, 512:1024], ps[:, 512:1024])
            nc.sync.dma_start(logits[bt * P : (bt + 1) * P, :], lg[:])
            if pending is not None:
                epi_sqrt(*pending)

            snb = epi_norm(bt, lg)

            dt_ = dpool.tile([P, C], out_dt)
            nc.scalar.activation(
                dt_[:, 0:512],
                lg[:, 0:512],
                mybir.ActivationFunctionType.Sqrt,
                bias=snb[:],
                scale=-2.0 * ALPHA,
            )
            nc.sync.dma_start(dist[bt * P : (bt + 1) * P, 0:512], dt_[:, 0:512])
            nc.scalar.activation(
                dt_[:, 512:1024],
                lg[:, 512:1024],
                mybir.ActivationFunctionType.Sqrt,
                bias=snb[:],
                scale=-2.0 * ALPHA,
            )
            x_eng.dma_start(
                dist[bt * P : (bt + 1) * P, 512:1024], dt_[:, 512:1024]
            )

        # ---- matmul schedule ----
        pss = [psum.tile([P, C], F32, tag="ps", name=f"ps{i}") for i in range(4)]

        # Dummy matmuls bridge preamble-end -> first-input-landed so HAM's
        # clock-gate releases before the real stream starts. They land in
        # pss[3]'s hi bank; b-tile 3's start=True group overwrites later.
        for _ in range(N_DUMMY):
            nc.tensor.matmul(
                pss[3][:, 512:1024],
                warm[:, 0:P],
                warm[:],
                start=True,
                stop=True,
                skip_group_check=True,
            )

        # k-major warm-in groups of 2 b-tiles: each (x_k, w_k) DMA piece
        # unlocks 4 matmuls, and the first epilogues fire early enough that
        # ACT/DVE epilogue work fits inside the matmul window. Both copies of
        # a group are emitted before the chains so ACT and DVE drain the two
        # PSUM buffers concurrently.
        for g0 in (0, 2):
            for k in range(KT):
                if g0 == 0 and k == 0:
                    # k0 runs lo-halves of both tiles first: they need only
                    # the first two DMA pieces (xA0's first half + w0lo), so
                    # the real stream starts ~1us before w0hi lands. Costs 2
                    # extra LDWEIGHTS.
                    for i in (0, 1):
                        nc.tensor.matmul(
                            pss[i][:, 0:512], x_slice(0, i), wt_lo[0],
                            start=True, stop=False,
                        )
                    for i in (0, 1):
                        nc.tensor.matmul(
                            pss[i][:, 512:1024], x_slice(0, i), wt_hi[0],
                            start=True, stop=False,
                        )
                    continue
                for i in (g0, g0 + 1):
                    mm(i, pss[i], k)
            lgs = [epi_copy(g0 + i, pss[g0 + i]) for i in range(2)]
            for i in range(2):
                epi_chain(g0 + i, lgs[i])

        pending = None
        for bt in range(4, NBT):
            ps = psum.tile([P, C], F32, tag="ps")
            for k in range(KT):
                mm(bt, ps, k)
            if not dve_copy:
                epilogue(bt, ps)
            elif bt == NBT - 2:
                # defer tile 14's sqrt past tile 15's copies (ACT in-order
                # queue would otherwise block the last tile's chain start)
                lg = epi_copy(bt, ps)
                pending = (bt, lg, epi_norm(bt, lg))
            elif bt == NBT - 1:
                epilogue_last(bt, ps, pending)
            else:
                epilogue(bt, ps)

    nc.compile()
    return nc


_NC = {}


def _round_tf32(a):
    """Round-to-nearest-even to TF32 (10-bit mantissa) in fp32 storage."""
    u = a.view(np.uint32)
    r = (u + np.uint32(0xFFF) + ((u >> np.uint32(13)) & np.uint32(1))) & np.uint32(
        0xFFFFE000
    )
    return r.view(np.float32)


def kernel(x, W, trace=False, _result_box=None, io_mode=IO_MODE, **flags):
    key = (io_mode, tuple(sorted(flags.items())))
    if key not in _NC:
        _NC[key] = build(io_mode, **flags)
    nc = _NC[key]

    x = np.ascontiguousarray(np.asarray(x, dtype=np.float32))
    W = np.ascontiguousarray(np.asarray(W, dtype=np.float32))
    if io_mode == "bf16":
        prep = lambda a: np.asarray(a, dtype=ml_dtypes.bfloat16)
    else:
        prep = _round_tf32
    wT = prep(np.ascontiguousarray(W.T))
    in_maps = [
        {
            "xT": prep(np.ascontiguousarray(x[i * BS : (i + 1) * BS, :].T)),
            "wT": wT,
        }
        for i in range(N_CORES)
    ]

    # The first execution of a freshly loaded NEFF has been seen to flake
    # (transient NRT_EXEC_UNIT_UNRECOVERABLE / corrupt output on this
    # fabric); do a throwaway warm-up exec with one retry, then the real run.
    try:
        run_bass_kernel_spmd(nc, in_maps, list(range(N_CORES)))
    except Exception:
        try:
            run_bass_kernel_spmd(nc, in_maps, list(range(N_CORES)))
        except Exception:
            pass

    res = run_bass_kernel_spmd(nc, in_maps, list(range(N_CORES)), trace=trace)
    if _result_box is not None:
        _result_box.append(res)

    logits = np.concatenate(
        [np.asarray(res.results[i]["logits"], dtype=np.float32) for i in range(N_CORES)],
        axis=0,
    )
    dist = np.concatenate(
        [np.asarray(res.results[i]["dist"], dtype=np.float32) for i in range(N_CORES)],
        axis=0,
    )
    return logits, dist


# revision 49
# speedup vs baseline: 1.0370x; 1.0036x over previous
"""Trainium2 Bass kernel for nn_CACProjector (logits = x @ W^T, CAC distances).

Strategy: data-parallel over batch B across 8 NeuronCores. Each core gets a
(768, 2048) column-slice xT of x^T (host-side transpose so the contraction
dim D lands on SBUF partitions) and a replicated W^T (768, 1024). On-core:

  logits[b, c] = sum_d xT[d, b] * wT[d, c]      (PE, fp32 accumulate in PSUM)
  sq_norm[b]   = sum_c logits[b, c]^2 + alpha^2 (DVE square+reduce+add on the
                                                 bf16 logits copy)
  dist[b, c]   = sqrt(sq_norm[b] - 2*alpha*logits[b, c])
                                                 (ACT Sqrt w/ scale+bias)

The PE runs at its bf16 roofline (192 N=512 matmuls x ~216 ns = 41.5 us);
everything else is scheduled to hide under it:

- All input loads ride the sync HWDGE ring in exact consumption order
  (k0 pieces split small so the first real matmul starts ~10.5 us).
  Splitting loads across both HWDGE rings was measured WORSE: the SDMA
  engine pool round-robins between rings, so parallel issue just halves
  per-transfer bandwidth and delays every early completion.
- The k-major warm-in groups are 2 b-tiles, so the first epilogues fire at
  ~17 us and the ACT/DVE epilogue work (~34 us each side) fits inside the
  matmul window instead of backlogging past its end.
- Per-tile epilogue: one engine copies PSUM->SBUF bf16 (the sole PSUM
  reader; DVE for DVE_COPY_TILES, else ACT), then DVE squares+reduces the
  copy (tensor_tensor_reduce crashes this fabric's exec unit - don't), then
  ACT Sqrt. PSUM banks free right at the copy, so the 4-buffer pool never
  stalls the PE.
- End of kernel: tile 14's sqrt is deferred in ACT program order past tile
  15's lo-half copy (the in-order ACT queue would otherwise head-of-line
  block the last chain); tile 15's copy halves run on ACT+DVE concurrently
  (different PSUM banks), and its dist halves store via both DMA rings to
  overlap the ~1.5 us store-receipt latency.
- 6 dummy matmuls on a zeroed tile (memset on GpSimd, whose preamble ends
  earliest) bridge NEFF-preamble-end (~7.3 us) to first-input-landed
  (~10.5 us) so the PE's HAM clock-gate (1.2 -> 2.4 GHz after ~3.4 us of
  sustained activity) is released when the real stream starts. A PE idle
  gap > ~1 us here re-throttles the clock for ~4 us of the stream.

d2 = ||l||^2 - 2a*l_j + a^2 >= (l_j - a)^2 >= 0 mathematically, and with this
data d2 ~ 1100 >> 0, so the reference's maximum(d2, 0) clamp is a no-op.

IO_MODE picks transport precision ("bf16" ships x/W/outputs bf16, fp32 PSUM
accumulate; "f32r" ships fp32 rounded to TF32).
"""

import sys

sys.path.insert(0, "/opt/trn_rl_repo")

from contextlib import ExitStack

import ml_dtypes
import numpy as np

import concourse.tile as tile
from concourse import bacc, mybir
from concourse.bass_utils import run_bass_kernel_spmd

N_CORES = 8
B, D, C = 16384, 768, 1024
BS = B // N_CORES          # 2048 rows of B per core
P = 128                    # partition dim
KT = D // P                # 6 contraction chunks
NBT = BS // P              # 16 output row-tiles per core
ALPHA = 10.0

F32 = mybir.dt.float32
F32R = mybir.dt.float32r
BF16 = mybir.dt.bfloat16

IO_MODE = "bf16"
# 6 dummies bridge preamble-end (~7.16us) toward first-input-landed
# (~10.1-10.6). A 7th (to close the residual 0.4us gap for HAM-window
# insurance) measured no better and delays the real stream when inputs land
# early — 6 won on measurement (64.5us vs 65.5).
N_DUMMY = 6
# tiles whose PSUM->SBUF copy runs on DVE (rest: ACT) — balances ACT/DVE.
# (1,3) measured best. STT + six DVE copies (to unload ~95%-occupied ACT)
# measured 67.1us vs ~64.5 for this config — reverted.
DVE_COPY_TILES = (1, 3, 5, 7, 9, 11)
# tiles whose square runs on GpSimd: measured 1.9us/op (2.7x DVE) and it
# serializes into every tile's chain — keep empty.
GPSIMD_SQ_TILES = frozenset()
# fused square+row-sum via scalar_tensor_tensor (one DVE pass instead of
# tensor_tensor + tensor_reduce): SCALAR_TENSOR_TENSOR 1224ns + 84ns
# DVE_READ_ACCUMULATOR, correct on HW, saves ~0.6us/tile of DVE occupancy —
# but measured WORSE end-to-end both alone (66.7us) and with rebalanced
# copies (67.1us) vs the unfused program (64.0-65.5us). Keep False.
USE_STT = True

# Feature flags (bisectable): scalar-ring DMA issue, fused square+reduce,
# DVE-side PSUM copies.
SCALAR_DMA = True
USE_TTR = False  # tensor_tensor_reduce crashes the exec unit on this fabric
DVE_COPY = True


def build(io_mode=IO_MODE, scalar_dma=None, use_ttr=None, dve_copy=None):
    scalar_dma = SCALAR_DMA if scalar_dma is None else scalar_dma
    use_ttr = USE_TTR if use_ttr is None else use_ttr
    dve_copy = DVE_COPY if dve_copy is None else dve_copy
    in_dt = BF16 if io_mode == "bf16" else F32R
    out_dt = BF16 if io_mode == "bf16" else F32
    MULT = mybir.AluOpType.mult
    ADD = mybir.AluOpType.add

    nc = bacc.Bacc("TRN2", target_bir_lowering=False, debug=False)
    xT = nc.dram_tensor("xT", [D, BS], in_dt, kind="ExternalInput").ap()
    wT = nc.dram_tensor("wT", [D, C], in_dt, kind="ExternalInput").ap()
    logits = nc.dram_tensor("logits", [BS, C], out_dt, kind="ExternalOutput").ap()
    dist = nc.dram_tensor("dist", [BS, C], out_dt, kind="ExternalOutput").ap()

    with tile.TileContext(nc) as tc, ExitStack() as ctx:
        xpool = ctx.enter_context(tc.tile_pool(name="xT", bufs=1))
        wpool = ctx.enter_context(tc.tile_pool(name="wT", bufs=1))
        psum = ctx.enter_context(tc.tile_pool(name="psum", bufs=4, space="PSUM"))
        lpool = ctx.enter_context(tc.tile_pool(name="lg", bufs=4))
        dpool = ctx.enter_context(tc.tile_pool(name="dist", bufs=4))
        spool = ctx.enter_context(tc.tile_pool(name="sq", bufs=2))
        npool = ctx.enter_context(tc.tile_pool(name="norms", bufs=4))

        # PE warmup fodder (no DMA dependency). memset on GpSimd: its
        # preamble ends ~6.3us vs Vector's ~7.0, so the dummy matmuls (and
        # with them the HAM clock-gate warmup window) start ~0.6us earlier.
        warm = xpool.tile([P, 512], in_dt, tag="warm")
        nc.gpsimd.memset(warm[:], 0)

        # ---- input loads, split across the two HWDGE rings ----
        # scalar (ACT) ring: x for b-tiles 0-3, then x for b-tiles 8-15
        # All input loads ride the sync ring only, in consumption order:
        # parallel-ring issue just splits SDMA bandwidth and delays every
        # early completion (measured: k1 pieces took 4.8us issue->complete
        # with two rings vs ~1us cadence single-ring). First k0 pieces are
        # small so the first real matmul can start ~10.3us.
        x_eng = nc.scalar if scalar_dma else nc.sync
        xA0 = xpool.tile([P, 512], in_dt, tag="xA0")
        nc.sync.dma_start(xA0[:, 0:256], xT[0:P, 0:256])
        w0lo = wpool.tile([P, 512], in_dt, tag="w0lo")
        nc.sync.dma_start(w0lo[:], wT[0:P, 0:512])
        w0hi = wpool.tile([P, 512], in_dt, tag="w0hi")
        nc.sync.dma_start(w0hi[:], wT[0:P, 512:1024])
        nc.sync.dma_start(xA0[:, 256:512], xT[0:P, 256:512])
        xA = [xA0]
        wt_lo, wt_hi = [w0lo[:]], [w0hi[:]]
        for k in range(1, KT):
            t = xpool.tile([P, 512], in_dt, tag=f"xA{k}")
            nc.sync.dma_start(t[:], xT[k * P : (k + 1) * P, 0:512])
            xA.append(t)
            wk = wpool.tile([P, C], in_dt, tag=f"w{k}")
            nc.sync.dma_start(wk[:], wT[k * P : (k + 1) * P, :])
            wt_lo.append(wk[:, 0:512])
            wt_hi.append(wk[:, 512:1024])
        xB = []
        for k in range(KT):
            t = xpool.tile([P, 512], in_dt, tag=f"xB{k}")
            nc.sync.dma_start(t[:], xT[k * P : (k + 1) * P, 512:1024])
            xB.append(t)
        # xC stays on the sync ring too. Moving it to the scalar ring was
        # tried with padding ops to delay its transfers past the critical
        # ramp window — but Tile schedules by dependency/priority, not
        # emission order, so the dependency-free xC issues ran FIRST and
        # contended with the ramp (+6us). Two concurrent rings halve each
        # transfer's bandwidth (SDMA packet round-robin).
        xC = []
        for k in range(KT):
            t = xpool.tile([P, 1024], in_dt, tag=f"xC{k}")
            nc.sync.dma_start(t[:], xT[k * P : (k + 1) * P, 1024:2048])
            xC.append(t)

        def x_slice(k, bt):
            if bt < 4:
                return xA[k][:, bt * P : (bt + 1) * P]
            if bt < 8:
                return xB[k][:, (bt - 4) * P : (bt - 3) * P]
            return xC[k][:, (bt - 8) * P : (bt - 7) * P]

        def mm(bt, ps, k):
            lhs = x_slice(k, bt)
            nc.tensor.matmul(
                ps[:, 0:512], lhs, wt_lo[k], start=(k == 0), stop=(k == KT - 1)
            )
            nc.tensor.matmul(
                ps[:, 512:1024], lhs, wt_hi[k], start=(k == 0), stop=(k == KT - 1)
            )

        def epi_copy(bt, ps):
            # The copy is the sole PSUM reader; DVE for tiles in
            # DVE_COPY_TILES (load balance + burst parallelism), ACT else.
            lg = lpool.tile([P, C], out_dt, name=f"lg{bt}")
            if dve_copy and bt in DVE_COPY_TILES:
                nc.vector.tensor_copy(lg[:], ps[:])
            else:
                nc.scalar.copy(lg[:], ps[:])
            nc.sync.dma_start(logits[bt * P : (bt + 1) * P, :], lg[:])
            return lg

        def epi_norm(bt, lg):
            # DVE: square + reduce + (+alpha^2) -> snb. scalar_tensor_tensor
            # fuses square+row-sum into ONE pass (InstTensorScalarPtr — a
            # different opcode family from the broken tensor_tensor_reduce).
            sq = spool.tile([P, C], out_dt, tag="sq")
            sn = npool.tile([P, 1], F32, tag="sn")
            snb = npool.tile([P, 1], F32, tag="snb")
            if USE_STT:
                nc.vector.scalar_tensor_tensor(
                    sq[:], lg[:], 1.0, lg[:], mybir.AluOpType.bypass, MULT,
                    accum_out=sn[:],
                )
            else:
                sq_eng = nc.gpsimd if bt in GPSIMD_SQ_TILES else nc.vector
                sq_eng.tensor_tensor(sq[:], lg[:], lg[:], MULT)
                nc.vector.tensor_reduce(
                    sn[:], sq[:], axis=mybir.AxisListType.X, op=ADD
                )
            nc.vector.tensor_scalar_add(snb[:], sn[:], ALPHA * ALPHA)
            return snb

        def epi_sqrt(bt, lg, snb):
            dt_ = dpool.tile([P, C], out_dt)
            nc.scalar.activation(
                dt_[:],
                lg[:],
                mybir.ActivationFunctionType.Sqrt,
                bias=snb[:],
                scale=-2.0 * ALPHA,
            )
            nc.sync.dma_start(dist[bt * P : (bt + 1) * P, :], dt_[:])

        def epi_chain(bt, lg):
            epi_sqrt(bt, lg, epi_norm(bt, lg))

        def epilogue(bt, ps):
            epi_chain(bt, epi_copy(bt, ps))

        def epilogue_last(bt, ps, pending):
            # End-of-kernel chain. Tile 14's sqrt was deferred (pending) so
            # ACT can run this tile's lo-half copy the moment the last matmul
            # retires (no head-of-line blocking behind sqrt14's DVE wait);
            # DVE copies the hi half concurrently (different PSUM bank). The
            # dist halves store via both DMA rings to overlap the final
            # receipt latency.
            lg = lpool.tile([P, C], out_dt, name=f"lg{bt}")
            nc.scalar.copy(lg[:, 0:512], ps[:, 0:512])
            nc.vector.tensor_copy(lg[:, 512:1024], ps[:, 512:1024])
            nc.sync.dma_start(logits[bt * P : (bt + 1) * P, :], lg[:])
            if pending is not None:
                epi_sqrt(*pending)

            snb = epi_norm(bt, lg)

            dt_ = dpool.tile([P, C], out_dt)
            nc.scalar.activation(
                dt_[:, 0:512],
                lg[:, 0:512],
                mybir.ActivationFunctionType.Sqrt,
                bias=snb[:],
                scale=-2.0 * ALPHA,
            )
            nc.sync.dma_start(dist[bt * P : (bt + 1) * P, 0:512], dt_[:, 0:512])
            nc.scalar.activation(
                dt_[:, 512:1024],
                lg[:, 512:1024],
                mybir.ActivationFunctionType.Sqrt,
                bias=snb[:],
                scale=-2.0 * ALPHA,
            )
            x_eng.dma_start(
                dist[bt * P : (bt + 1) * P, 512:1024], dt_[:, 512:1024]
            )

        # ---- matmul schedule ----
        pss = [psum.tile([P, C], F32, tag="ps", name=f"ps{i}") for i in range(4)]

        # Dummy matmuls bridge preamble-end -> first-input-landed so HAM's
        # clock-gate releases before the real stream starts. They land in
        # pss[3]'s hi bank; b-tile 3's start=True group overwrites later.
        for _ in range(N_DUMMY):
            nc.tensor.matmul(
                pss[3][:, 512:1024],
                warm[:, 0:P],
                warm[:],
                start=True,
                stop=True,
                skip_group_check=True,
            )

        # k-major warm-in groups of 2 b-tiles: each (x_k, w_k) DMA piece
        # unlocks 4 matmuls, and the first epilogues fire early enough that
        # ACT/DVE epilogue work fits inside the matmul window. Both copies of
        # a group are emitted before the chains so ACT and DVE drain the two
        # PSUM buffers concurrently.
        for g0 in (0, 2):
            for k in range(KT):
                if g0 == 0 and k == 0:
                    # k0 runs lo-halves of both tiles first: they need only
                    # the first two DMA pieces (xA0's first half + w0lo), so
                    # the real stream starts ~1us before w0hi lands. Costs 2
                    # extra LDWEIGHTS.
                    for i in (0, 1):
                        nc.tensor.matmul(
                            pss[i][:, 0:512], x_slice(0, i), wt_lo[0],
                            start=True, stop=False,
                        )
                    for i in (0, 1):
                        nc.tensor.matmul(
                            pss[i][:, 512:1024], x_slice(0, i), wt_hi[0],
                            start=True, stop=False,
                        )
                    continue
                for i in (g0, g0 + 1):
                    mm(i, pss[i], k)
            lgs = [epi_copy(g0 + i, pss[g0 + i]) for i in range(2)]
            for i in range(2):
                epi_chain(g0 + i, lgs[i])

        pending = None
        for bt in range(4, NBT):
            ps = psum.tile([P, C], F32, tag="ps")
            for k in range(KT):
                mm(bt, ps, k)
            if not dve_copy:
                epilogue(bt, ps)
            elif bt == NBT - 2:
                # defer tile 14's sqrt past tile 15's copies (ACT in-order
                # queue would otherwise block the last tile's chain start)
                lg = epi_copy(bt, ps)
                pending = (bt, lg, epi_norm(bt, lg))
            elif bt == NBT - 1:
                epilogue_last(bt, ps, pending)
            else:
                epilogue(bt, ps)

    nc.compile()
    return nc


_NC = {}


def _round_tf32(a):
    """Round-to-nearest-even to TF32 (10-bit mantissa) in fp32 storage."""
    u = a.view(np.uint32)
    r = (u + np.uint32(0xFFF) + ((u >> np.uint32(13)) & np.uint32(1))) & np.uint32(
        0xFFFFE000
    )
    return r.view(np.float32)


def kernel(x, W, trace=False, _result_box=None, io_mode=IO_MODE, **flags):
    key = (io_mode, tuple(sorted(flags.items())))
    if key not in _NC:
        _NC[key] = build(io_mode, **flags)
    nc = _NC[key]

    x = np.ascontiguousarray(np.asarray(x, dtype=np.float32))
    W = np.ascontiguousarray(np.asarray(W, dtype=np.float32))
    if io_mode == "bf16":
        prep = lambda a: np.asarray(a, dtype=ml_dtypes.bfloat16)
    else:
        prep = _round_tf32
    wT = prep(np.ascontiguousarray(W.T))
    in_maps = [
        {
            "xT": prep(np.ascontiguousarray(x[i * BS : (i + 1) * BS, :].T)),
            "wT": wT,
        }
        for i in range(N_CORES)
    ]

    # The first execution of a freshly loaded NEFF has been seen to flake
    # (transient NRT_EXEC_UNIT_UNRECOVERABLE / corrupt output on this
    # fabric); do a throwaway warm-up exec with one retry, then the real run.
    try:
        run_bass_kernel_spmd(nc, in_maps, list(range(N_CORES)))
    except Exception:
        try:
            run_bass_kernel_spmd(nc, in_maps, list(range(N_CORES)))
        except Exception:
            pass

    res = run_bass_kernel_spmd(nc, in_maps, list(range(N_CORES)), trace=trace)
    if _result_box is not None:
        _result_box.append(res)

    logits = np.concatenate(
        [np.asarray(res.results[i]["logits"], dtype=np.float32) for i in range(N_CORES)],
        axis=0,
    )
    dist = np.concatenate(
        [np.asarray(res.results[i]["dist"], dtype=np.float32) for i in range(N_CORES)],
        axis=0,
    )
    return logits, dist
